# Initial kernel scaffold
#
"""Trainium2 Bass kernel for nn_ConvolutionalSelfAttention.

Mathematical simplification of the reference:
    v[b,t,o]  = sum_c x[b,t,c] W_attn[o,c]
    s[b,t]    = sum_o v[b,t,o] = sum_c x[b,t,c] * wa[c],   wa = colsum(W_attn)
    y[b,t]    = (s[b,t] + s[b,t-1] + s[b,t-2]) / 3        (zero-padded, causal)
    out[b,t,o]= y[b,t] * wp[o],                            wp = rowsum(W_proj)

Sharding (8 cores): each core owns 2048 consecutive tokens of one batch
(batch b = core//2, half = core%2) plus a 2-token halo.  Weight
reductions are sharded: core k reduces rows [256k,256k+256) of W_attn
(PE ones-matmul colsum) and columns [256k,256k+256) of W_proj (DVE
free-dim rowsum); a single 16KB AllReduce produces the full wa/wp
vectors on every core.

Per 128-token tile on device:
    s_col  = tensor_tensor_reduce(x_tile * wa_bcast) / 3        (DVE, fused)
    y_col  = M1^T @ s_col + M2^T @ s_prev_col                   (PE band matmul,
             handles the cross-partition causal shifts)
    out    = wp_bcast scaled per-partition by y_col             (ACT activation)
"""

import numpy as np
from contextlib import ExitStack

B, T, C = 4, 4096, 2048
N_CORES = 8
T_LOC = (B * T) // N_CORES      # 2048 tokens per core
P = 128
NT = T_LOC // P                 # 16 token tiles per core
CH = 2                          # token tiles per DMA chunk
NCH = NT // CH                  # 8 chunks
RSL = C // N_CORES              # 256 weight rows/cols per core
GW = C // P                     # 16 row-groups in a [C, RSL] slice
THIRD = 1.0 / 3.0

_BUILT = {}


def _band_consts():
    # lhsT layout: y[m] = sum_k M[k, m] * s[k]
    m1 = np.zeros((P, P), np.float32)
    for m in range(P):
        m1[max(0, m - 2):m + 1, m] = 1.0
    # carry from previous tile's last two tokens (partitions 126/127)
    m2 = np.zeros((P, P), np.float32)
    m2[126, 0] = 1.0
    m2[127, 0] = 1.0
    m2[127, 1] = 1.0
    # carry for tile 0: halo column stores s[-2], s[-1] at partitions 0/1
    # (compute engines can only address partition starts 0/32/64/96, so the
    # halo dot products are computed at partitions 0-1)
    m2h = np.zeros((P, P), np.float32)
    m2h[0, 0] = 1.0
    m2h[1, 0] = 1.0
    m2h[1, 1] = 1.0
    return m1, m2, m2h


def _build_nc():
    import concourse.bass as bass
    import concourse.tile as tile
    from concourse import bacc, mybir

    f32 = mybir.dt.float32
    AF = mybir.ActivationFunctionType
    ALU = mybir.AluOpType

    nc = bacc.Bacc("TRN2", target_bir_lowering=False, debug=False,
                   num_devices=N_CORES)

    x_shard = nc.dram_tensor("x_shard", [T_LOC, C], f32, kind="ExternalInput")
    x_halo = nc.dram_tensor("x_halo", [2, C], f32, kind="ExternalInput")
    wa_rows = nc.dram_tensor("wa_rows", [RSL, C], f32, kind="ExternalInput")
    wp_cols = nc.dram_tensor("wp_cols", [C, RSL], f32, kind="ExternalInput")
    out = nc.dram_tensor("out", [T_LOC, C], f32, kind="ExternalOutput")

    m1_c = nc.dram_tensor("m1_band", [P, P], f32, kind="ExternalInput")
    m2_c = nc.dram_tensor("m2_band", [P, P], f32, kind="ExternalInput")
    m2h_c = nc.dram_tensor("m2h_band", [P, P], f32, kind="ExternalInput")
    ones_k_c = nc.dram_tensor("ones_k", [P, 1], f32, kind="ExternalInput")
    ones_m_c = nc.dram_tensor("ones_m", [1, P], f32, kind="ExternalInput")
    ident_c = nc.dram_tensor("ident128", [P, P], f32, kind="ExternalInput")

    with tile.TileContext(nc) as tc, ExitStack() as ctx:
        cpool = ctx.enter_context(tc.tile_pool(name="const", bufs=1))
        wrow_pool = ctx.enter_context(tc.tile_pool(name="wrow", bufs=2))
        wbig_pool = ctx.enter_context(tc.tile_pool(name="wbig", bufs=1))
        hpool = ctx.enter_context(tc.tile_pool(name="halo", bufs=1))
        xpool = ctx.enter_context(tc.tile_pool(name="x", bufs=3))
        opool = ctx.enter_context(tc.tile_pool(name="o", bufs=2))
        spool = ctx.enter_context(tc.tile_pool(name="small", bufs=2))
        scratch = ctx.enter_context(tc.tile_pool(name="scratch", bufs=2))
        pre_psum = ctx.enter_context(tc.tile_pool(name="prepsum", bufs=1, space="PSUM"))
        ypsum = ctx.enter_context(tc.tile_pool(name="ypsum", bufs=2, space="PSUM"))
        dram = ctx.enter_context(tc.tile_pool(name="dram", bufs=1, space="DRAM"))

        # ---- constants into SBUF
        m1_sb = cpool.tile([P, P], f32)
        nc.gpsimd.dma_start(m1_sb[:], m1_c.ap())
        m2_sb = cpool.tile([P, P], f32)
        nc.gpsimd.dma_start(m2_sb[:], m2_c.ap())
        m2h_sb = cpool.tile([P, P], f32)
        nc.gpsimd.dma_start(m2h_sb[:], m2h_c.ap())
        ones_k = cpool.tile([P, 1], f32)
        nc.gpsimd.dma_start(ones_k[:], ones_k_c.ap())
        ones_m = cpool.tile([1, P], f32)
        nc.gpsimd.dma_start(ones_m[:], ones_m_c.ap())
        id_sb = cpool.tile([P, P], f32)
        nc.gpsimd.dma_start(id_sb[:], ident_c.ap())

        cc_in = dram.tile([2, C], f32)
        cc_out = dram.tile([2, C], f32)


        # ---- W_attn colsum partial: ones^T @ rows -> [1, C]
        wa_tiles = []
        for u in range(RSL // P):
            wt = wrow_pool.tile([P, C], f32)
            nc.gpsimd.dma_start(wt[:], wa_rows.ap()[u * P:(u + 1) * P, :])
            wa_tiles.append(wt)
        wa_row = spool.tile([1, C], f32)
        for j in range(C // 512):
            pj = pre_psum.tile([1, 512], f32)
            for u in range(RSL // P):
                nc.tensor.matmul(
                    pj[:], lhsT=ones_k[:], rhs=wa_tiles[u][:, j * 512:(j + 1) * 512],
                    start=(u == 0), stop=(u == RSL // P - 1))
            nc.scalar.copy(wa_row[0:1, j * 512:(j + 1) * 512], pj[:])
        nc.gpsimd.dma_start(cc_in[0:1, :], wa_row[:])

        # ---- W_proj col-slice rowsum partial -> [C] (o-ordered via PE transpose)
        wp_big = wbig_pool.tile([P, GW * RSL], f32)
        nc.gpsimd.dma_start(
            wp_big[:].rearrange("p (g r) -> p g r", g=GW),
            wp_cols.ap().rearrange("(g p) r -> p g r", p=P))
        wp_t = spool.tile([P, GW], f32)
        nc.vector.tensor_reduce(
            wp_t[:], wp_big[:].rearrange("p (g r) -> p g r", g=GW),
            axis=mybir.AxisListType.X, op=ALU.add)
        wpt_ps = pre_psum.tile([GW, P], f32)
        nc.tensor.transpose(wpt_ps[:], wp_t[:], id_sb[:])
        wpt_row = spool.tile([GW, P], f32)
        nc.scalar.copy(wpt_row[:], wpt_ps[:])
        nc.gpsimd.dma_start(
            cc_in[1:2, :].rearrange("a (u p) -> (a u) p", u=GW), wpt_row[:])

        # ---- one AllReduce for both reduced vectors
        nc.gpsimd.collective_compute(
            "AllReduce", ALU.add,
            replica_groups=[list(range(N_CORES))],
            ins=[cc_in[:].opt()], outs=[cc_out[:].opt()])

        war_sb = spool.tile([1, C], f32)
        nc.gpsimd.dma_start(war_sb[:], cc_out[0:1, :])
        wpr_sb = spool.tile([1, C], f32)
        nc.gpsimd.dma_start(wpr_sb[:], cc_out[1:2, :])

        # ---- broadcast [1, C] rows to all 128 partitions via K=1 PE matmul
        wa_bcast = cpool.tile([P, C], f32)
        wp_bcast = cpool.tile([P, C], f32)
        for j in range(C // 512):
            bp = pre_psum.tile([P, 512], f32)
            nc.tensor.matmul(bp[:], lhsT=ones_m[:],
                             rhs=war_sb[0:1, j * 512:(j + 1) * 512],
                             start=True, stop=True)
            nc.scalar.mul(wa_bcast[:, j * 512:(j + 1) * 512], bp[:], THIRD)
            bq = pre_psum.tile([P, 512], f32)
            nc.tensor.matmul(bq[:], lhsT=ones_m[:],
                             rhs=wpr_sb[0:1, j * 512:(j + 1) * 512],
                             start=True, stop=True)
            nc.scalar.copy(wp_bcast[:, j * 512:(j + 1) * 512], bq[:])

        # ---- s_all[:, 0] = halo column (partitions 0/1 = s[-2], s[-1])
        s_all = cpool.tile([P, NT + 1], f32)
        nc.vector.memset(s_all[:, 0:1], 0.0)
        ht = hpool.tile([P, C], f32)
        nc.gpsimd.dma_start(ht[0:2, :], x_halo.ap())
        scr_h = scratch.tile([P, C], f32, tag="scr")
        nc.vector.tensor_mul(scr_h[0:2, :], ht[0:2, :], wa_bcast[0:2, :])
        nc.vector.tensor_reduce(
            s_all[0:2, 0:1], scr_h[0:2, :], axis=mybir.AxisListType.X, op=ALU.add)

        # ---- main loop: chunked stream over 16 token tiles
        for ch in range(NCH):
            xc = xpool.tile([P, CH * C], f32)
            nc.gpsimd.dma_start(
                xc[:].rearrange("p (h c) -> p h c", h=CH),
                x_shard.ap()[ch * CH * P:(ch + 1) * CH * P, :]
                .rearrange("(h p) c -> p h c", p=P))
            oc = opool.tile([P, CH * C], f32)
            for h in range(CH):
                i = ch * CH + h
                scr = scratch.tile([P, C], f32, tag="scr")
                nc.vector.tensor_mul(scr[:], xc[:, h * C:(h + 1) * C], wa_bcast[:])
                nc.vector.tensor_reduce(
                    s_all[:, i + 1:i + 2], scr[:], axis=mybir.AxisListType.X,
                    op=ALU.add)
                yp = ypsum.tile([P, 1], f32)
                nc.tensor.matmul(yp[:], lhsT=m1_sb[:], rhs=s_all[:, i + 1:i + 2],
                                 start=True, stop=False)
                carry = m2h_sb if i == 0 else m2_sb
                nc.tensor.matmul(yp[:], lhsT=carry[:], rhs=s_all[:, i:i + 1],
                                 start=False, stop=True)
                ysb = spool.tile([P, 1], f32)
                nc.vector.tensor_copy(ysb[:], yp[:])
                nc.scalar.activation(oc[:, h * C:(h + 1) * C], wp_bcast[:],
                                     AF.Copy, scale=ysb[:, 0:1])
            nc.gpsimd.dma_start(
                out.ap()[ch * CH * P:(ch + 1) * CH * P, :]
                .rearrange("(h p) c -> p h c", p=P),
                oc[:].rearrange("p (h c) -> p h c", h=CH))

    nc.compile()
    return nc


def _get_nc():
    if "nc" not in _BUILT:
        _BUILT["nc"] = _build_nc()
    return _BUILT["nc"]


def make_in_maps(x, W_attn, W_proj):
    x = np.ascontiguousarray(np.asarray(x, dtype=np.float32))
    W_attn = np.ascontiguousarray(np.asarray(W_attn, dtype=np.float32))
    W_proj = np.ascontiguousarray(np.asarray(W_proj, dtype=np.float32))
    m1_np, m2_np, m2h_np = _band_consts()
    consts = {
        "m1_band": m1_np, "m2_band": m2_np, "m2h_band": m2h_np,
        "ones_k": np.ones((P, 1), np.float32),
        "ones_m": np.ones((1, P), np.float32),
        "ident128": np.eye(P, dtype=np.float32),
    }
    in_maps = []
    for k in range(N_CORES):
        b, h = divmod(k, 2)
        t0 = h * T_LOC
        if h == 0:
            halo = np.zeros((2, C), np.float32)
        else:
            halo = np.ascontiguousarray(x[b, t0 - 2:t0, :])
        in_maps.append({
            "x_shard": np.ascontiguousarray(x[b, t0:t0 + T_LOC, :]),
            "x_halo": halo,
            "wa_rows": np.ascontiguousarray(W_attn[k * RSL:(k + 1) * RSL, :]),
            "wp_cols": np.ascontiguousarray(W_proj[:, k * RSL:(k + 1) * RSL]),
            **consts,
        })
    return in_maps


def assemble(results):
    out_full = np.empty((B, T, C), np.float32)
    for k in range(N_CORES):
        b, h = divmod(k, 2)
        t0 = h * T_LOC
        out_full[b, t0:t0 + T_LOC, :] = results[k]["out"]
    return out_full


def kernel(x, W_attn, W_proj):
    from concourse.bass_utils import run_bass_kernel_spmd

    nc = _get_nc()
    in_maps = make_in_maps(x, W_attn, W_proj)
    res = run_bass_kernel_spmd(nc, in_maps, list(range(N_CORES)))
    return assemble(res.results)



# revision 11
# speedup vs baseline: 1.8520x; 1.8520x over previous
"""Trainium2 Bass kernel for nn_ConvolutionalSelfAttention.

Mathematical simplification of the reference:
    v[b,t,o]  = sum_c x[b,t,c] W_attn[o,c]
    s[b,t]    = sum_o v[b,t,o] = sum_c x[b,t,c] * wa[c],   wa = colsum(W_attn)
    y[b,t]    = (s[b,t] + s[b,t-1] + s[b,t-2]) / 3        (zero-padded, causal)
    out[b,t,o]= y[b,t] * wp[o],                            wp = rowsum(W_proj)

Sharding (8 cores, collective-free): each core owns 2048 consecutive
tokens of one batch (b = core//2, half = core%2) plus a 2-token halo.
Every core computes the full wa/wp weight reductions locally from
bf16 copies of W_attn and W_proj^T (host-transposed), so there is no
AllReduce and no cross-core barrier: cores run fully independently.

All HBM streams are bf16 (x, W_attn, W_proj^T, out); the harness gate
is scale-relative absmax, and bf16 keeps the error ~5e-3 << 2e-2.

Per 128-token tile on device:
    s_col = tensor_reduce(tensor_mul(x_tile, wa_bcast/3))      (DVE, two pass;
            tensor_tensor_reduce hangs on this HW path - do not use it)
    y_col = M1^T @ s_col + M2^T @ s_prev_col                   (PE band matmul)
    out   = wp_bcast scaled per-partition by y_col -> bf16     (ACT activation)

Weight reduction: DVE in-place add chain over 16 row-tiles -> [128, C] acc,
one PE ones-matmul colsum -> [1, C], PE K=1 broadcast -> [128, C].
All DMAs ride gpsimd SWDGE (HWDGE dma_start also failed on this path).
"""

import numpy as np
from contextlib import ExitStack

B, T, C = 4, 4096, 2048
N_CORES = 8
T_LOC = (B * T) // N_CORES      # 2048 tokens per core
P = 128
NT = T_LOC // P                 # 16 token tiles per core
GW = C // P                     # 16 row-tiles in a [C, C] matrix
WCH = 4                         # row-tiles per weight DMA chunk (2 MB bf16)
NWC = GW // WCH                 # 4 chunks per matrix
XCH = 4                         # token tiles per x/out DMA chunk (2 MB bf16)
NXC = NT // XCH                 # 4 chunks
THIRD = 1.0 / 3.0

_BUILT = {}


def _band_consts():
    # lhsT layout: y[m] = sum_k M[k, m] * s[k]
    m1 = np.zeros((P, P), np.float32)
    for m in range(P):
        m1[max(0, m - 2):m + 1, m] = 1.0
    # carry from previous tile's last two tokens (partitions 126/127)
    m2 = np.zeros((P, P), np.float32)
    m2[126, 0] = 1.0
    m2[127, 0] = 1.0
    m2[127, 1] = 1.0
    # carry for tile 0: halo column stores s[-2], s[-1] at partitions 0/1
    m2h = np.zeros((P, P), np.float32)
    m2h[0, 0] = 1.0
    m2h[1, 0] = 1.0
    m2h[1, 1] = 1.0
    return m1, m2, m2h


def _build_nc():
    import concourse.bass as bass
    import concourse.tile as tile
    from concourse import bacc, mybir

    f32 = mybir.dt.float32
    bf16 = mybir.dt.bfloat16
    AF = mybir.ActivationFunctionType
    ALU = mybir.AluOpType

    nc = bacc.Bacc("TRN2", target_bir_lowering=False, debug=False,
                   num_devices=N_CORES)

    x_shard = nc.dram_tensor("x_shard", [T_LOC, C], bf16, kind="ExternalInput")
    x_halo = nc.dram_tensor("x_halo", [2, C], bf16, kind="ExternalInput")
    w_attn = nc.dram_tensor("w_attn", [C, C], bf16, kind="ExternalInput")
    w_projT = nc.dram_tensor("w_projT", [C, C], bf16, kind="ExternalInput")
    out = nc.dram_tensor("out", [T_LOC, C], bf16, kind="ExternalOutput")

    m1_c = nc.dram_tensor("m1_band", [P, P], f32, kind="ExternalInput")
    m2_c = nc.dram_tensor("m2_band", [P, P], f32, kind="ExternalInput")
    m2h_c = nc.dram_tensor("m2h_band", [P, P], f32, kind="ExternalInput")
    ones_k_c = nc.dram_tensor("ones_k", [P, 1], bf16, kind="ExternalInput")
    ones_m_c = nc.dram_tensor("ones_m", [1, P], f32, kind="ExternalInput")

    with tile.TileContext(nc) as tc, ExitStack() as ctx:
        cpool = ctx.enter_context(tc.tile_pool(name="const", bufs=1))
        wchunk = ctx.enter_context(tc.tile_pool(name="wchunk", bufs=3))
        xchunk = ctx.enter_context(tc.tile_pool(name="xchunk", bufs=3))
        opool = ctx.enter_context(tc.tile_pool(name="o", bufs=2))
        scratch = ctx.enter_context(tc.tile_pool(name="scratch", bufs=2))
        spool = ctx.enter_context(tc.tile_pool(name="small", bufs=4))
        psA = ctx.enter_context(tc.tile_pool(name="psA", bufs=2, space="PSUM"))
        psB = ctx.enter_context(tc.tile_pool(name="psB", bufs=2, space="PSUM"))
        ypsum = ctx.enter_context(tc.tile_pool(name="ypsum", bufs=4, space="PSUM"))

        # ---- constants into SBUF
        m1_sb = cpool.tile([P, P], f32)
        nc.gpsimd.dma_start(m1_sb[:], m1_c.ap())
        m2_sb = cpool.tile([P, P], f32)
        nc.gpsimd.dma_start(m2_sb[:], m2_c.ap())
        m2h_sb = cpool.tile([P, P], f32)
        nc.gpsimd.dma_start(m2h_sb[:], m2h_c.ap())
        ones_k = cpool.tile([P, 1], bf16)
        nc.gpsimd.dma_start(ones_k[:], ones_k_c.ap())
        ones_m = cpool.tile([1, P], f32)
        nc.gpsimd.dma_start(ones_m[:], ones_m_c.ap())
        ht = cpool.tile([2, C], bf16)
        nc.gpsimd.dma_start(ht[:], x_halo.ap())

        def reduce_weight(dram_t, scale):
            """colsum of a [C, C] bf16 matrix -> bf16 [P, C] broadcast*scale."""
            acc = cpool.tile([P, C], bf16, tag="wacc_" + dram_t.name)
            for c in range(NWC):
                wc = wchunk.tile([P, WCH * C], bf16, tag="wc")
                nc.gpsimd.dma_start(
                    wc[:].rearrange("p (h c) -> p h c", h=WCH),
                    dram_t.ap()[c * WCH * P:(c + 1) * WCH * P, :]
                    .rearrange("(h p) c -> p h c", p=P))
                if c == 0:
                    nc.vector.tensor_tensor(
                        acc[:], wc[:, 0:C], wc[:, C:2 * C], ALU.add)
                else:
                    nc.vector.tensor_tensor(acc[:], acc[:], wc[:, 0:C], ALU.add)
                    nc.vector.tensor_tensor(acc[:], acc[:], wc[:, C:2 * C], ALU.add)
                nc.vector.tensor_tensor(acc[:], acc[:], wc[:, 2 * C:3 * C], ALU.add)
                nc.vector.tensor_tensor(acc[:], acc[:], wc[:, 3 * C:4 * C], ALU.add)
            row = cpool.tile([1, C], f32, tag="wrow_" + dram_t.name)
            for j in range(C // 512):
                pj = psA.tile([1, 512], f32)
                nc.tensor.matmul(pj[:], lhsT=ones_k[:],
                                 rhs=acc[:, j * 512:(j + 1) * 512],
                                 start=True, stop=True)
                nc.scalar.copy(row[0:1, j * 512:(j + 1) * 512], pj[:])
            bcast = cpool.tile([P, C], bf16, tag="wbc_" + dram_t.name)
            for j in range(C // 512):
                bp = psB.tile([P, 512], f32)
                nc.tensor.matmul(bp[:], lhsT=ones_m[:],
                                 rhs=row[0:1, j * 512:(j + 1) * 512],
                                 start=True, stop=True)
                if scale == 1.0:
                    nc.scalar.copy(bcast[:, j * 512:(j + 1) * 512], bp[:])
                else:
                    nc.scalar.mul(bcast[:, j * 512:(j + 1) * 512], bp[:], scale)
            return bcast

        wa_bcast = reduce_weight(w_attn, THIRD)   # wa/3, bf16 [P, C]
        wp_bcast = reduce_weight(w_projT, 1.0)    # wp,   bf16 [P, C]

        # ---- halo s values: partitions 0/1 of a zeroed [P, 1] column
        s_halo = cpool.tile([P, 1], f32)
        nc.vector.memset(s_halo[:], 0.0)
        scr_h = scratch.tile([P, C], bf16, tag="scr")
        scr2_h = scratch.tile([P, C], bf16, tag="scr2")
        nc.vector.tensor_mul(scr_h[0:2, :], ht[0:2, :], wa_bcast[0:2, :])
        # free-dim sum via ACT accum_out (DVE tensor_reduce is capped at 1x
        # and would bottleneck; the copy output is discarded)
        nc.scalar.activation(scr2_h[0:2, :], scr_h[0:2, :], AF.Copy,
                             accum_out=s_halo[0:2, 0:1])

        # ---- main loop: stream 16 token tiles in 4 chunks
        s_prev = s_halo
        for ch in range(NXC):
            xc = xchunk.tile([P, XCH * C], bf16, tag="xc")
            nc.gpsimd.dma_start(
                xc[:].rearrange("p (h c) -> p h c", h=XCH),
                x_shard.ap()[ch * XCH * P:(ch + 1) * XCH * P, :]
                .rearrange("(h p) c -> p h c", p=P))
            oc = opool.tile([P, XCH * C], bf16, tag="oc")
            for h in range(XCH):
                i = ch * XCH + h
                scr = scratch.tile([P, C], bf16, tag="scr")
                scr2 = scratch.tile([P, C], bf16, tag="scr2")
                s_cur = spool.tile([P, 1], f32, tag="scol")
                nc.vector.tensor_mul(scr[:], xc[:, h * C:(h + 1) * C],
                                     wa_bcast[:])
                nc.scalar.activation(scr2[:], scr[:], AF.Copy,
                                     accum_out=s_cur[:])
                yp = ypsum.tile([P, 1], f32)
                nc.tensor.matmul(yp[:], lhsT=m1_sb[:], rhs=s_cur[:],
                                 start=True, stop=False)
                carry = m2h_sb if i == 0 else m2_sb
                nc.tensor.matmul(yp[:], lhsT=carry[:], rhs=s_prev[:],
                                 start=False, stop=True)
                ysb = spool.tile([P, 1], f32, tag="ycol")
                nc.vector.tensor_copy(ysb[:], yp[:])
                # out tile on DVE tensor_scalar (4x bf16 tier, ~0.7us) to keep
                # ACT free for the accum reductions
                nc.vector.tensor_scalar_mul(oc[:, h * C:(h + 1) * C],
                                            wp_bcast[:], ysb[:, 0:1])
                s_prev = s_cur
            nc.gpsimd.dma_start(
                out.ap()[ch * XCH * P:(ch + 1) * XCH * P, :]
                .rearrange("(h p) c -> p h c", p=P),
                oc[:].rearrange("p (h c) -> p h c", h=XCH))

    nc.compile()
    return nc


def _get_nc():
    if "nc" not in _BUILT:
        _BUILT["nc"] = _build_nc()
    return _BUILT["nc"]


def make_in_maps(x, W_attn, W_proj):
    import ml_dtypes
    bf = ml_dtypes.bfloat16
    x = np.asarray(x, dtype=np.float32)
    wa_bf = np.ascontiguousarray(np.asarray(W_attn, dtype=np.float32)).astype(bf)
    wpT_bf = np.ascontiguousarray(
        np.asarray(W_proj, dtype=np.float32).T).astype(bf)
    x_bf = x.astype(bf)
    m1_np, m2_np, m2h_np = _band_consts()
    consts = {
        "m1_band": m1_np, "m2_band": m2_np, "m2h_band": m2h_np,
        "ones_k": np.ones((P, 1), bf),
        "ones_m": np.ones((1, P), np.float32),
        "w_attn": wa_bf,
        "w_projT": wpT_bf,
    }
    in_maps = []
    for k in range(N_CORES):
        b, h = divmod(k, 2)
        t0 = h * T_LOC
        if h == 0:
            halo = np.zeros((2, C), bf)
        else:
            halo = np.ascontiguousarray(x_bf[b, t0 - 2:t0, :])
        in_maps.append({
            "x_shard": np.ascontiguousarray(x_bf[b, t0:t0 + T_LOC, :]),
            "x_halo": halo,
            **consts,
        })
    return in_maps


def assemble(results):
    out_full = np.empty((B, T, C), np.float32)
    for k in range(N_CORES):
        b, h = divmod(k, 2)
        t0 = h * T_LOC
        out_full[b, t0:t0 + T_LOC, :] = np.asarray(
            results[k]["out"], dtype=np.float32)
    return out_full


def kernel(x, W_attn, W_proj):
    from concourse.bass_utils import run_bass_kernel_spmd

    nc = _get_nc()
    in_maps = make_in_maps(x, W_attn, W_proj)
    res = run_bass_kernel_spmd(nc, in_maps, list(range(N_CORES)))
    return assemble(res.results)
